# revision 8
# baseline (speedup 1.0000x reference)
"""DCPNet rigid-alignment head on 8 Trainium2 NeuronCores.

Data-parallel over batch: B=16 samples -> 2 per core. Per sample the device
computes, in one fused pipeline:
  pd[m,n]  = ||se_n||^2 - 2 te_m . se_n + ||te_m||^2   (as one PE accumulation:
             4 K-chunks of the embedding matmul + 1 augmented K=2 matmul that
             adds -0.5*xx[n] and -0.5*yy[m]; pd = -2 * psum)
  d        = sqrt(pd) = exp(0.5 * ln(pd))     (ACT, single ln/exp table set)
  E        = exp(-d)                          (unnormalized softmax weights)
  C[n,:]   = [sum_m E[m,n]*tgt_m | sum_m E[m,n]]   (PE matmul with ones col)
  corr     = C[:,0:3] / C[:,3]                (soft correspondences)
  out44    = [srcs|1]^T-style 4x4 moment matrix: H_raw, N*src_mean,
             N*corr_mean, N                   (PE matmul over n-chunks)
The host does only the per-sample 3x3 SVD -> R, t, euler angles (16 tiny
matrices).
"""

import sys

if "/opt/trn_rl_repo" not in sys.path:
    sys.path.insert(0, "/opt/trn_rl_repo")

import numpy as np

_B, _N, _D = 16, 1024, 512
_NCORES = 8
_SPC = _B // _NCORES  # samples per core

_state = {}


def _build():
    if "nc" in _state:
        return _state["nc"]

    from contextlib import ExitStack

    import concourse.tile as tile
    from concourse import bacc
    from concourse import mybir
    from concourse.masks import make_identity

    fp32 = mybir.dt.float32
    f32r = mybir.dt.float32r
    AF = mybir.ActivationFunctionType

    KC = _D // 128  # 4 contraction chunks
    MC = _N // 128  # 8 partition chunks of the score matrix
    NH = _N // 512  # 2 free-dim halves

    nc = bacc.Bacc()
    srcs = nc.declare_dram_parameter("srcs", [_SPC, 3, _N], fp32, isOutput=False)
    tgts = nc.declare_dram_parameter("tgts", [_SPC, 3, _N], fp32, isOutput=False)
    semb = nc.declare_dram_parameter("srcs_emb", [_SPC, _D, _N], fp32, isOutput=False)
    temb = nc.declare_dram_parameter("tgts_emb", [_SPC, _D, _N], fp32, isOutput=False)
    out44 = nc.declare_dram_parameter("out44", [_SPC, 4, 4], fp32, isOutput=True)

    with ExitStack() as ctx:
        tc = ctx.enter_context(tile.TileContext(nc))
        singles = ctx.enter_context(tc.tile_pool(name="singles", bufs=1))
        emb = ctx.enter_context(tc.tile_pool(name="emb", bufs=2))
        sqp = ctx.enter_context(tc.tile_pool(name="sqp", bufs=2))
        work = ctx.enter_context(tc.tile_pool(name="work", bufs=3))
        small = ctx.enter_context(tc.tile_pool(name="small", bufs=2))
        psg = ctx.enter_context(tc.tile_pool(name="psg", bufs=2, space="PSUM"))
        psc = ctx.enter_context(tc.tile_pool(name="psc", bufs=2, space="PSUM"))
        pss = ctx.enter_context(tc.tile_pool(name="pss", bufs=3, space="PSUM"))

        ident = singles.tile([4, 4], fp32)
        make_identity(nc, ident)
        neghalf = singles.tile([128, 1], f32r)
        nc.vector.memset(neghalf.bitcast(fp32), -0.5)

        for s in range(_SPC):
            # ---- load embeddings as [128, KC, N] (k-chunks on partition) ----
            se_t = emb.tile([128, KC, _N], f32r, tag="se")
            te_t = emb.tile([128, KC, _N], f32r, tag="te")
            nc.sync.dma_start(out=se_t, in_=semb[s].rearrange("(k p) n -> p k n", p=128).bitcast(f32r))
            nc.sync.dma_start(out=te_t, in_=temb[s].rearrange("(k p) n -> p k n", p=128).bitcast(f32r))

            # ---- transposed point clouds with ones column ----
            # srcsT_aug[:, q, :] = [srcs_x srcs_y srcs_z 1] for n-chunk q
            srcsT_aug = small.tile([128, MC, 4], f32r, tag="srcsT")
            tgtsT_aug = small.tile([128, MC, 4], f32r, tag="tgtsT")
            nc.vector.memset(srcsT_aug.bitcast(fp32), 1.0)
            nc.vector.memset(tgtsT_aug.bitcast(fp32), 1.0)
            srcs_nd = srcs[s].rearrange("d n -> n d").bitcast(f32r)
            tgts_nd = tgts[s].rearrange("d n -> n d").bitcast(f32r)
            for q in range(MC):
                nc.sync.dma_start(
                    out=srcsT_aug[:, q, 0:3], in_=srcs_nd[q * 128 : (q + 1) * 128, :]
                )
                nc.sync.dma_start(
                    out=tgtsT_aug[:, q, 0:3], in_=tgts_nd[q * 128 : (q + 1) * 128, :]
                )

            # ---- xx = |se_n|^2, yy = |te_m|^2 as -0.5-scaled rows ----
            # Pairing in the augmented matmul (sum_k lhsT[k,m]*rhs[k,n]):
            #   k=0: lhsT = -0.5*yy[m], rhs = 1   -> adds -0.5*yy[m]
            #   k=1: lhsT = 1,          rhs = -0.5*xx[n] -> adds -0.5*xx[n]
            # Data rows that DVE can't write (partition 1) are filled via DMA.
            aug_lhsT = small.tile([2, _N], f32r, tag="auglhs")
            aug_rhs = small.tile([2, _N], f32r, tag="augrhs")
            nc.vector.memset(aug_lhsT.bitcast(fp32), 1.0)
            nc.vector.memset(aug_rhs.bitcast(fp32), 1.0)
            for emb_t, dst_row, use_dma in (
                (se_t, aug_rhs, True),  # xx -> aug_rhs row 1 (DMA)
                (te_t, aug_lhsT, False),  # yy -> aug_lhsT row 0 (DVE)
            ):
                red = [
                    pss.tile([1, 512], fp32, tag="ps1", name=f"red{h}")
                    for h in range(NH)
                ]
                for k in range(KC):
                    sq = sqp.tile([128, _N], f32r, tag="sq")
                    nc.vector.tensor_mul(sq, emb_t[:, k, :], emb_t[:, k, :])
                    for h in range(NH):
                        nc.tensor.matmul(
                            red[h],
                            neghalf,
                            sq[:, h * 512 : (h + 1) * 512],
                            start=(k == 0),
                            stop=(k == KC - 1),
                        )
                if use_dma:
                    xsc = small.tile([1, _N], f32r, tag="xsc")
                    for h in range(NH):
                        hsl = slice(h * 512, (h + 1) * 512)
                        nc.vector.tensor_copy(xsc[:, hsl], red[h])
                    nc.sync.dma_start(out=dst_row[1:2, :], in_=xsc)
                else:
                    for h in range(NH):
                        hsl = slice(h * 512, (h + 1) * 512)
                        nc.vector.tensor_copy(dst_row[0:1, hsl], red[h])

            # ---- main pipeline over (n-half, m-chunk) ----
            corr_all = small.tile([128, MC, 4], f32r, tag="corr")
            nc.vector.memset(corr_all.bitcast(fp32), 1.0)
            for nh in range(NH):
                nsl = slice(nh * 512, (nh + 1) * 512)
                c_ps = psc.tile([4, 512], fp32, tag="cps")
                for m in range(MC):
                    msl = slice(m * 128, (m + 1) * 128)
                    g_ps = psg.tile([128, 512], fp32, tag="gps")
                    for k in range(KC):
                        nc.tensor.matmul(
                            g_ps,
                            te_t[:, k, msl],
                            se_t[:, k, nsl],
                            start=(k == 0),
                            stop=False,
                        )
                    nc.tensor.matmul(
                        g_ps,
                        aug_lhsT[:, msl],
                        aug_rhs[:, nsl],
                        start=False,
                        stop=True,
                    )
                    # d = sqrt(-2*g) = exp(0.5*ln(-2*g)); E = exp(-d)
                    d_t = work.tile([128, 512], fp32, tag="dt")
                    e_t = work.tile([128, 512], f32r, tag="et")
                    nc.scalar.activation(out=d_t, in_=g_ps, func=AF.Ln, scale=-2.0)
                    nc.scalar.activation(out=d_t, in_=d_t, func=AF.Exp, scale=0.5)
                    nc.scalar.activation(out=e_t, in_=d_t, func=AF.Exp, scale=-1.0)
                    nc.tensor.matmul(
                        c_ps,
                        tgtsT_aug[:, m, :],
                        e_t,
                        start=(m == 0),
                        stop=(m == MC - 1),
                    )
                # C rows: [Cx Cy Cz ssum] over this n-half; normalize per n
                c_sb = small.tile([4, 512], fp32, tag="csb")
                nc.vector.tensor_copy(c_sb, c_ps)
                for q in range(4):
                    ct_ps = pss.tile([128, 4], fp32, tag="ps1")
                    nc.tensor.transpose(ct_ps, c_sb[:, q * 128 : (q + 1) * 128], ident)
                    rs = small.tile([128, 1], fp32, tag="rs")
                    nc.vector.reciprocal(rs, ct_ps[:, 3:4])
                    nc.vector.tensor_scalar(
                        out=corr_all[:, nh * 4 + q, 0:3],
                        in0=ct_ps[:, 0:3],
                        scalar1=rs,
                        scalar2=None,
                        op0=mybir.AluOpType.mult,
                    )

            # ---- out44 = sum_n [src|1]_n [corr|1]_n^T ----
            o_ps = pss.tile([4, 4], fp32, tag="ps1")
            for q in range(MC):
                nc.tensor.matmul(
                    o_ps,
                    srcsT_aug[:, q, :],
                    corr_all[:, q, :],
                    start=(q == 0),
                    stop=(q == MC - 1),
                )
            o_sb = small.tile([4, 4], fp32, tag="osb")
            nc.vector.tensor_copy(o_sb, o_ps)
            nc.sync.dma_start(out=out44[s], in_=o_sb)

    nc.finalize()
    _state["nc"] = nc
    return nc


def _postprocess(o44):
    """o44: [B, 4, 4] moment matrices -> [B, 6] (euler angles, translation)."""
    B = o44.shape[0]
    o = o44.astype(np.float64)
    H_raw = o[:, 0:3, 0:3]
    ssum = o[:, 0:3, 3]
    csum = o[:, 3, 0:3]
    cnt = o[:, 3, 3][:, None, None]
    H = H_raw - ssum[:, :, None] * csum[:, None, :] / cnt
    u, _, vh = np.linalg.svd(H)
    v = np.swapaxes(vh, -1, -2)
    r = v @ np.swapaxes(u, -1, -2)
    det = np.linalg.det(r)
    flip = np.where(det[:, None] < 0, np.array([1.0, 1.0, -1.0]), 1.0)
    v = v * flip[:, None, :]
    R = v @ np.swapaxes(u, -1, -2)
    sm = ssum / cnt[:, :, 0]
    cm = csum / cnt[:, :, 0]
    t = -np.einsum("bij,bj->bi", R, sm) + cm
    cy = np.sqrt(R[:, 2, 2] ** 2 + R[:, 1, 2] ** 2)
    ax = np.arctan2(-R[:, 1, 2], R[:, 2, 2])
    ay = np.arctan2(R[:, 0, 2], cy)
    az = np.arctan2(-R[:, 0, 1], R[:, 0, 0])
    return np.concatenate([np.stack([ax, ay, az], 1), t], axis=1).astype(np.float32)


def kernel(srcs, tgts, srcs_emb, tgts_emb, **run_kwargs):
    from concourse.bass_utils import run_bass_kernel_spmd

    nc = _build()
    in_maps = []
    for c in range(_NCORES):
        sl = slice(c * _SPC, (c + 1) * _SPC)
        in_maps.append(
            {
                "srcs": np.ascontiguousarray(srcs[sl], dtype=np.float32),
                "tgts": np.ascontiguousarray(tgts[sl], dtype=np.float32),
                "srcs_emb": np.ascontiguousarray(srcs_emb[sl], dtype=np.float32),
                "tgts_emb": np.ascontiguousarray(tgts_emb[sl], dtype=np.float32),
            }
        )
    res = run_bass_kernel_spmd(nc, in_maps, list(range(_NCORES)), **run_kwargs)
    o44 = np.concatenate(
        [np.asarray(res.results[c]["out44"]) for c in range(_NCORES)], axis=0
    )
    out = _postprocess(o44)
    if run_kwargs:
        _state["last_results"] = res
    return out


# revision 10
# speedup vs baseline: 1.3724x; 1.3724x over previous
"""DCPNet rigid-alignment head on 8 Trainium2 NeuronCores.

Data-parallel over batch: B=16 samples -> 2 per core. Per sample the device
computes, in one fused pipeline:
  pd[m,n]  = ||se_n||^2 - 2 te_m . se_n + ||te_m||^2   (as one PE accumulation:
             4 K-chunks of the embedding matmul + 1 augmented K=2 matmul that
             adds -0.5*xx[n] and -0.5*yy[m]; pd = -2 * psum)
  d        = sqrt(pd) = exp(0.5 * ln(pd))     (ACT, single ln/exp table set)
  E        = exp(-d)                          (unnormalized softmax weights)
  C[n,:]   = [sum_m E[m,n]*tgt_m | sum_m E[m,n]]   (PE matmul with ones col)
  corr     = C[:,0:3] / C[:,3]                (soft correspondences)
  out44    = 4x4 moment matrix [H_raw, N*src_mean; N*corr_mean, N]
             (PE matmul over n-chunks of [src|1] x [corr|1])
The host does only the per-sample 3x3 SVD -> R, t, euler angles (16 tiny
matrices).

Matmuls run as float32r (full-rate reduced-precision fp32). ACT ops are
[128, 1024] (two PSUM banks per tile) to amortize fixed overhead, and all
transcendentals live in the natural_log_exp_and_others table set so there
is exactly one ACT_TABLE_LOAD in the whole kernel.
"""

import sys

if "/opt/trn_rl_repo" not in sys.path:
    sys.path.insert(0, "/opt/trn_rl_repo")

import numpy as np

_B, _N, _D = 16, 1024, 512
_NCORES = 8
_SPC = _B // _NCORES  # samples per core

_state = {}


def _patch_act_tables():
    """Make natural_log_exp_and_others the only set providing Ln/Exp so the
    table-load inserter never thrashes between the ln-only and exp-only sets
    (each switch costs ~2.7us and this kernel alternates Ln/Exp per tile)."""
    from concourse import bacc, hw_specs, mybir

    if getattr(bacc, "_dcp_act_patch", False):
        return
    orig = hw_specs.get_activation_tables

    def patched(module_arch):
        tables = dict(orig(module_arch))
        ln = mybir.ActivationFunctionType.Ln
        ex = mybir.ActivationFunctionType.Exp
        for name, funcs in tables.items():
            if name != "natural_log_exp_and_others":
                funcs.discard(ln)
                funcs.discard(ex)
        return tables

    bacc.get_activation_tables = patched
    bacc._dcp_act_patch = True


def _build():
    if "nc" in _state:
        return _state["nc"]

    from contextlib import ExitStack

    import concourse.tile as tile
    from concourse import bacc, mybir
    from concourse.masks import make_identity

    _patch_act_tables()

    fp32 = mybir.dt.float32
    f32r = mybir.dt.float32r
    AF = mybir.ActivationFunctionType

    KC = _D // 128  # 4 contraction chunks
    MC = _N // 128  # 8 partition chunks of the score matrix
    NH = _N // 512  # 2 free-dim halves

    nc = bacc.Bacc()
    srcs = nc.declare_dram_parameter("srcs", [_SPC, 3, _N], fp32, isOutput=False)
    tgts = nc.declare_dram_parameter("tgts", [_SPC, 3, _N], fp32, isOutput=False)
    semb = nc.declare_dram_parameter("srcs_emb", [_SPC, _D, _N], fp32, isOutput=False)
    temb = nc.declare_dram_parameter("tgts_emb", [_SPC, _D, _N], fp32, isOutput=False)
    out44 = nc.declare_dram_parameter("out44", [_SPC, 4, 4], fp32, isOutput=True)

    with ExitStack() as ctx:
        tc = ctx.enter_context(tile.TileContext(nc))
        singles = ctx.enter_context(tc.tile_pool(name="singles", bufs=1))
        emb = ctx.enter_context(tc.tile_pool(name="emb", bufs=2))
        sqp = ctx.enter_context(tc.tile_pool(name="sqp", bufs=2))
        work = ctx.enter_context(tc.tile_pool(name="work", bufs=3))
        small = ctx.enter_context(tc.tile_pool(name="small", bufs=2))
        # PSUM budget (8 banks): g2 tiles 2 banks x 2 bufs, c2 2 banks x 1,
        # small psums 1 bank x 2.
        psg = ctx.enter_context(tc.tile_pool(name="psg", bufs=2, space="PSUM"))
        psc = ctx.enter_context(tc.tile_pool(name="psc", bufs=1, space="PSUM"))
        pss = ctx.enter_context(tc.tile_pool(name="pss", bufs=2, space="PSUM"))

        ident = singles.tile([4, 4], fp32)
        make_identity(nc, ident)
        neghalf = singles.tile([128, 1], f32r)
        nc.vector.memset(neghalf.bitcast(fp32), -0.5)

        # per-sample persistent tiles
        se_t, te_t, srcsT_aug, tgtsT_aug, aug_lhsT, aug_rhs = (
            [None] * _SPC for _ in range(6)
        )

        # ---- phase 1 (both samples): loads + xx/yy reductions ----
        for s in range(_SPC):
            se_t[s] = emb.tile([128, KC, _N], f32r, tag="se", name=f"se{s}")
            te_t[s] = emb.tile([128, KC, _N], f32r, tag="te", name=f"te{s}")
            nc.sync.dma_start(
                out=se_t[s],
                in_=semb[s].rearrange("(k p) n -> p k n", p=128).bitcast(f32r),
            )
            nc.sync.dma_start(
                out=te_t[s],
                in_=temb[s].rearrange("(k p) n -> p k n", p=128).bitcast(f32r),
            )

            srcsT_aug[s] = small.tile([128, MC, 4], f32r, tag="srcsT", name=f"sT{s}")
            tgtsT_aug[s] = small.tile([128, MC, 4], f32r, tag="tgtsT", name=f"tT{s}")
            nc.vector.memset(srcsT_aug[s].bitcast(fp32), 1.0)
            nc.vector.memset(tgtsT_aug[s].bitcast(fp32), 1.0)
            srcs_nd = srcs[s].rearrange("d n -> n d").bitcast(f32r)
            tgts_nd = tgts[s].rearrange("d n -> n d").bitcast(f32r)
            for q in range(MC):
                nc.sync.dma_start(
                    out=srcsT_aug[s][:, q, 0:3],
                    in_=srcs_nd[q * 128 : (q + 1) * 128, :],
                )
                nc.sync.dma_start(
                    out=tgtsT_aug[s][:, q, 0:3],
                    in_=tgts_nd[q * 128 : (q + 1) * 128, :],
                )

            # augmented K=2 rows: see pairing note in the module docstring
            aug_lhsT[s] = small.tile([2, _N], f32r, tag="auglhs", name=f"al{s}")
            aug_rhs[s] = small.tile([2, _N], f32r, tag="augrhs", name=f"ar{s}")
            nc.vector.memset(aug_lhsT[s].bitcast(fp32), 1.0)
            nc.vector.memset(aug_rhs[s].bitcast(fp32), 1.0)
            for emb_t, dst_row, use_dma in (
                (se_t[s], aug_rhs[s], True),  # xx -> aug_rhs row 1 (via DMA)
                (te_t[s], aug_lhsT[s], False),  # yy -> aug_lhsT row 0 (DVE)
            ):
                red = [
                    pss.tile([1, 512], fp32, tag="ps1", name=f"red{s}{h}")
                    for h in range(NH)
                ]
                for k in range(KC):
                    sq = sqp.tile([128, _N], f32r, tag="sq", name=f"sq{s}{k}")
                    nc.vector.tensor_mul(sq, emb_t[:, k, :], emb_t[:, k, :])
                    for h in range(NH):
                        nc.tensor.matmul(
                            red[h],
                            neghalf,
                            sq[:, h * 512 : (h + 1) * 512],
                            start=(k == 0),
                            stop=(k == KC - 1),
                        )
                if use_dma:
                    xsc = small.tile([1, _N], f32r, tag="xsc", name=f"xsc{s}")
                    for h in range(NH):
                        nc.vector.tensor_copy(xsc[:, h * 512 : (h + 1) * 512], red[h])
                    nc.sync.dma_start(out=dst_row[1:2, :], in_=xsc)
                else:
                    for h in range(NH):
                        nc.vector.tensor_copy(
                            dst_row[0:1, h * 512 : (h + 1) * 512], red[h]
                        )

        # ---- phase 2 (per sample): scores -> E -> C ----
        for s in range(_SPC):
            c2 = psc.tile([4, NH, 512], fp32, tag="c2", name=f"c2_{s}")
            for m in range(MC):
                msl = slice(m * 128, (m + 1) * 128)
                g2 = psg.tile([128, NH, 512], fp32, tag="g2", name=f"g2_{s}{m}")
                for nh in range(NH):
                    nsl = slice(nh * 512, (nh + 1) * 512)
                    for k in range(KC):
                        nc.tensor.matmul(
                            g2[:, nh, :],
                            te_t[s][:, k, msl],
                            se_t[s][:, k, nsl],
                            start=(k == 0),
                            stop=False,
                        )
                    nc.tensor.matmul(
                        g2[:, nh, :],
                        aug_lhsT[s][:, msl],
                        aug_rhs[s][:, nsl],
                        start=False,
                        stop=True,
                    )
                # d = sqrt(-2*g) = exp(0.5*ln(-2*g)); E = exp(-d)
                d_t = work.tile([128, NH * 512], fp32, tag="dt", name=f"d{s}{m}")
                e_t = work.tile([128, NH * 512], f32r, tag="et", name=f"e{s}{m}")
                nc.scalar.activation(out=d_t, in_=g2.rearrange("p a b -> p (a b)"),
                                     func=AF.Ln, scale=-2.0)
                nc.scalar.activation(out=d_t, in_=d_t, func=AF.Exp, scale=0.5)
                nc.scalar.activation(out=e_t, in_=d_t, func=AF.Exp, scale=-1.0)
                for nh in range(NH):
                    nc.tensor.matmul(
                        c2[:, nh, :],
                        tgtsT_aug[s][:, m, :],
                        e_t[:, nh * 512 : (nh + 1) * 512],
                        start=(m == 0),
                        stop=(m == MC - 1),
                    )

            # ---- per-sample tail: normalize, moment matrix, store ----
            c_sb = small.tile([4, NH, 512], fp32, tag="csb", name=f"csb{s}")
            nc.vector.tensor_copy(c_sb, c2)
            corr_all = small.tile([128, MC, 4], f32r, tag="corr", name=f"corr{s}")
            nc.vector.memset(corr_all.bitcast(fp32), 1.0)
            c_flat = c_sb.rearrange("p a b -> p (a b)")
            for q in range(MC):
                ct_ps = pss.tile([128, 4], fp32, tag="ps1", name=f"ct{s}{q}")
                nc.tensor.transpose(ct_ps, c_flat[:, q * 128 : (q + 1) * 128], ident)
                rs = small.tile([128, 1], fp32, tag="rs", name=f"rs{s}{q}")
                nc.vector.reciprocal(rs, ct_ps[:, 3:4])
                nc.vector.tensor_scalar(
                    out=corr_all[:, q, 0:3],
                    in0=ct_ps[:, 0:3],
                    scalar1=rs,
                    scalar2=None,
                    op0=mybir.AluOpType.mult,
                )
            o_ps = pss.tile([4, 4], fp32, tag="ps1", name=f"o{s}")
            for q in range(MC):
                nc.tensor.matmul(
                    o_ps,
                    srcsT_aug[s][:, q, :],
                    corr_all[:, q, :],
                    start=(q == 0),
                    stop=(q == MC - 1),
                )
            o_sb = small.tile([4, 4], fp32, tag="osb", name=f"ot{s}")
            nc.vector.tensor_copy(o_sb, o_ps)
            nc.sync.dma_start(out=out44[s], in_=o_sb)

    nc.finalize()
    _state["nc"] = nc
    return nc


def _postprocess(o44):
    """o44: [B, 4, 4] moment matrices -> [B, 6] (euler angles, translation)."""
    o = o44.astype(np.float64)
    H_raw = o[:, 0:3, 0:3]
    ssum = o[:, 0:3, 3]
    csum = o[:, 3, 0:3]
    cnt = o[:, 3, 3][:, None, None]
    H = H_raw - ssum[:, :, None] * csum[:, None, :] / cnt
    u, _, vh = np.linalg.svd(H)
    v = np.swapaxes(vh, -1, -2)
    r = v @ np.swapaxes(u, -1, -2)
    det = np.linalg.det(r)
    flip = np.where(det[:, None] < 0, np.array([1.0, 1.0, -1.0]), 1.0)
    v = v * flip[:, None, :]
    R = v @ np.swapaxes(u, -1, -2)
    sm = ssum / cnt[:, :, 0]
    cm = csum / cnt[:, :, 0]
    t = -np.einsum("bij,bj->bi", R, sm) + cm
    cy = np.sqrt(R[:, 2, 2] ** 2 + R[:, 1, 2] ** 2)
    ax = np.arctan2(-R[:, 1, 2], R[:, 2, 2])
    ay = np.arctan2(R[:, 0, 2], cy)
    az = np.arctan2(-R[:, 0, 1], R[:, 0, 0])
    return np.concatenate([np.stack([ax, ay, az], 1), t], axis=1).astype(np.float32)


def kernel(srcs, tgts, srcs_emb, tgts_emb, **run_kwargs):
    from concourse.bass_utils import run_bass_kernel_spmd

    nc = _build()
    in_maps = []
    for c in range(_NCORES):
        sl = slice(c * _SPC, (c + 1) * _SPC)
        in_maps.append(
            {
                "srcs": np.ascontiguousarray(srcs[sl], dtype=np.float32),
                "tgts": np.ascontiguousarray(tgts[sl], dtype=np.float32),
                "srcs_emb": np.ascontiguousarray(srcs_emb[sl], dtype=np.float32),
                "tgts_emb": np.ascontiguousarray(tgts_emb[sl], dtype=np.float32),
            }
        )
    res = run_bass_kernel_spmd(nc, in_maps, list(range(_NCORES)), **run_kwargs)
    o44 = np.concatenate(
        [np.asarray(res.results[c]["out44"]) for c in range(_NCORES)], axis=0
    )
    out = _postprocess(o44)
    if run_kwargs:
        _state["last_results"] = res
    return out


# revision 12
# speedup vs baseline: 1.5241x; 1.1105x over previous
"""DCPNet rigid-alignment head on 8 Trainium2 NeuronCores.

Data-parallel over batch: B=16 samples -> 2 per core. Per sample the device
computes, in one fused pipeline:
  pd[m,n]  = ||se_n||^2 - 2 te_m . se_n + ||te_m||^2   (as one PE accumulation:
             4 K-chunks of the embedding matmul + 1 augmented K=2 matmul that
             adds -0.5*xx[n] and -0.5*yy[m]; pd = -2 * psum)
  d        = sqrt(pd) = exp(0.5 * ln(pd))     (ACT, single ln/exp table set)
  E        = exp(-d)                          (unnormalized softmax weights)
  C[n,:]   = [sum_m E[m,n]*tgt_m | sum_m E[m,n]]   (PE matmul with ones col)
  corr     = C[:,0:3] / C[:,3]                (soft correspondences)
  out44    = 4x4 moment matrix [H_raw, N*src_mean; N*corr_mean, N]
             (PE matmul over n-chunks of [src|1] x [corr|1])
The host does only the per-sample 3x3 SVD -> R, t, euler angles (16 tiny
matrices).

Matmuls run as float32r (full-rate reduced-precision fp32). ACT ops are
[128, 1024] (two PSUM banks per tile) to amortize fixed overhead, and all
transcendentals live in the natural_log_exp_and_others table set so there
is exactly one ACT_TABLE_LOAD in the whole kernel.
"""

import sys

if "/opt/trn_rl_repo" not in sys.path:
    sys.path.insert(0, "/opt/trn_rl_repo")

import numpy as np

_B, _N, _D = 16, 1024, 512
_NCORES = 8
_SPC = _B // _NCORES  # samples per core

_state = {}


def _patch_act_tables():
    """Make natural_log_exp_and_others the only set providing Ln/Exp so the
    table-load inserter never thrashes between the ln-only and exp-only sets
    (each switch costs ~2.7us and this kernel alternates Ln/Exp per tile)."""
    from concourse import bacc, hw_specs, mybir

    if getattr(bacc, "_dcp_act_patch", False):
        return
    orig = hw_specs.get_activation_tables

    def patched(module_arch):
        tables = dict(orig(module_arch))
        ln = mybir.ActivationFunctionType.Ln
        ex = mybir.ActivationFunctionType.Exp
        for name, funcs in tables.items():
            if name != "natural_log_exp_and_others":
                funcs.discard(ln)
                funcs.discard(ex)
        return tables

    bacc.get_activation_tables = patched
    bacc._dcp_act_patch = True


def _enable_ldw_opt():
    """Flip walrus's --enable-ldw-opt to true: with the k-outer/nh-inner loop
    order below, consecutive G matmuls share their stationary operand, and the
    LDWEIGHTS dedup halves the serialized 4-byte weight-load tax."""
    from concourse import bass_utils

    if getattr(bass_utils, "_dcp_ldw_patch", False):
        return
    orig = bass_utils.run_command

    def patched(cmd, *a, **kw):
        if isinstance(cmd, list):
            cmd = [
                "--enable-ldw-opt=true" if c == "--enable-ldw-opt=false" else c
                for c in cmd
            ]
        return orig(cmd, *a, **kw)

    bass_utils.run_command = patched
    bass_utils._dcp_ldw_patch = True


def _build():
    if "nc" in _state:
        return _state["nc"]

    from contextlib import ExitStack

    import concourse.tile as tile
    from concourse import bacc, mybir
    from concourse.masks import make_identity

    _patch_act_tables()
    _enable_ldw_opt()

    fp32 = mybir.dt.float32
    f32r = mybir.dt.float32r
    AF = mybir.ActivationFunctionType

    KC = _D // 128  # 4 contraction chunks
    MC = _N // 128  # 8 partition chunks of the score matrix
    NH = _N // 512  # 2 free-dim halves

    nc = bacc.Bacc()
    srcs = nc.declare_dram_parameter("srcs", [_SPC, 3, _N], fp32, isOutput=False)
    tgts = nc.declare_dram_parameter("tgts", [_SPC, 3, _N], fp32, isOutput=False)
    semb = nc.declare_dram_parameter("srcs_emb", [_SPC, _D, _N], fp32, isOutput=False)
    temb = nc.declare_dram_parameter("tgts_emb", [_SPC, _D, _N], fp32, isOutput=False)
    out44 = nc.declare_dram_parameter("out44", [_SPC, 4, 4], fp32, isOutput=True)

    with ExitStack() as ctx:
        tc = ctx.enter_context(tile.TileContext(nc))
        singles = ctx.enter_context(tc.tile_pool(name="singles", bufs=1))
        emb = ctx.enter_context(tc.tile_pool(name="emb", bufs=2))
        sqp = ctx.enter_context(tc.tile_pool(name="sqp", bufs=2))
        work = ctx.enter_context(tc.tile_pool(name="work", bufs=3))
        small = ctx.enter_context(tc.tile_pool(name="small", bufs=2))
        # PSUM budget (8 banks): g2 tiles 2 banks x 2 bufs, c2 2 banks x 1,
        # small psums 1 bank x 2.
        psg = ctx.enter_context(tc.tile_pool(name="psg", bufs=2, space="PSUM"))
        psc = ctx.enter_context(tc.tile_pool(name="psc", bufs=1, space="PSUM"))
        pss = ctx.enter_context(tc.tile_pool(name="pss", bufs=2, space="PSUM"))

        ident = singles.tile([4, 4], fp32)
        make_identity(nc, ident)
        neghalf = singles.tile([128, 1], f32r)
        nc.vector.memset(neghalf.bitcast(fp32), -0.5)

        # per-sample persistent tiles
        se_t, te_t, srcsT_aug, tgtsT_aug, aug_lhsT, aug_rhs = (
            [None] * _SPC for _ in range(6)
        )

        # ---- phase 1 (both samples): loads + xx/yy reductions ----
        for s in range(_SPC):
            se_t[s] = emb.tile([128, KC, _N], f32r, tag="se", name=f"se{s}")
            te_t[s] = emb.tile([128, KC, _N], f32r, tag="te", name=f"te{s}")
            nc.sync.dma_start(
                out=se_t[s],
                in_=semb[s].rearrange("(k p) n -> p k n", p=128).bitcast(f32r),
            )
            nc.sync.dma_start(
                out=te_t[s],
                in_=temb[s].rearrange("(k p) n -> p k n", p=128).bitcast(f32r),
            )

            srcsT_aug[s] = small.tile([128, MC, 4], f32r, tag="srcsT", name=f"sT{s}")
            tgtsT_aug[s] = small.tile([128, MC, 4], f32r, tag="tgtsT", name=f"tT{s}")
            nc.vector.memset(srcsT_aug[s].bitcast(fp32), 1.0)
            nc.vector.memset(tgtsT_aug[s].bitcast(fp32), 1.0)
            srcs_nd = srcs[s].rearrange("d n -> n d").bitcast(f32r)
            tgts_nd = tgts[s].rearrange("d n -> n d").bitcast(f32r)
            for q in range(MC):
                nc.sync.dma_start(
                    out=srcsT_aug[s][:, q, 0:3],
                    in_=srcs_nd[q * 128 : (q + 1) * 128, :],
                )
                nc.sync.dma_start(
                    out=tgtsT_aug[s][:, q, 0:3],
                    in_=tgts_nd[q * 128 : (q + 1) * 128, :],
                )

            # augmented K=2 rows: see pairing note in the module docstring
            aug_lhsT[s] = small.tile([2, _N], f32r, tag="auglhs", name=f"al{s}")
            aug_rhs[s] = small.tile([2, _N], f32r, tag="augrhs", name=f"ar{s}")
            nc.vector.memset(aug_lhsT[s].bitcast(fp32), 1.0)
            nc.vector.memset(aug_rhs[s].bitcast(fp32), 1.0)
            for emb_t, dst_row, use_dma in (
                (se_t[s], aug_rhs[s], True),  # xx -> aug_rhs row 1 (via DMA)
                (te_t[s], aug_lhsT[s], False),  # yy -> aug_lhsT row 0 (DVE)
            ):
                red = [
                    pss.tile([1, 512], fp32, tag="ps1", name=f"red{s}{h}")
                    for h in range(NH)
                ]
                for k in range(KC):
                    sq = sqp.tile([128, _N], f32r, tag="sq", name=f"sq{s}{k}")
                    nc.vector.tensor_mul(sq, emb_t[:, k, :], emb_t[:, k, :])
                    for h in range(NH):
                        nc.tensor.matmul(
                            red[h],
                            neghalf,
                            sq[:, h * 512 : (h + 1) * 512],
                            start=(k == 0),
                            stop=(k == KC - 1),
                        )
                if use_dma:
                    xsc = small.tile([1, _N], f32r, tag="xsc", name=f"xsc{s}")
                    for h in range(NH):
                        nc.vector.tensor_copy(xsc[:, h * 512 : (h + 1) * 512], red[h])
                    nc.sync.dma_start(out=dst_row[1:2, :], in_=xsc)
                else:
                    for h in range(NH):
                        nc.vector.tensor_copy(
                            dst_row[0:1, h * 512 : (h + 1) * 512], red[h]
                        )

        # ---- phase 2 (per sample): scores -> E -> C ----
        for s in range(_SPC):
            c2 = psc.tile([4, NH, 512], fp32, tag="c2", name=f"c2_{s}")
            for m in range(MC):
                msl = slice(m * 128, (m + 1) * 128)
                g2 = psg.tile([128, NH, 512], fp32, tag="g2", name=f"g2_{s}{m}")
                # k outer, nh inner: consecutive matmuls share the stationary
                # operand so walrus's LDWEIGHTS dedup can elide every other
                # (expensive, 4-byte) weight load.
                for k in range(KC):
                    for nh in range(NH):
                        nc.tensor.matmul(
                            g2[:, nh, :],
                            te_t[s][:, k, msl],
                            se_t[s][:, k, nh * 512 : (nh + 1) * 512],
                            start=(k == 0),
                            stop=False,
                        )
                for nh in range(NH):
                    nc.tensor.matmul(
                        g2[:, nh, :],
                        aug_lhsT[s][:, msl],
                        aug_rhs[s][:, nh * 512 : (nh + 1) * 512],
                        start=False,
                        stop=True,
                    )
                # d = sqrt(-2*g) = exp(0.5*ln(-2*g)); E = exp(-d)
                d_t = work.tile([128, NH * 512], fp32, tag="dt", name=f"d{s}{m}")
                e_t = work.tile([128, NH * 512], f32r, tag="et", name=f"e{s}{m}")
                nc.scalar.activation(out=d_t, in_=g2.rearrange("p a b -> p (a b)"),
                                     func=AF.Ln, scale=-2.0)
                nc.scalar.activation(out=d_t, in_=d_t, func=AF.Exp, scale=0.5)
                nc.scalar.activation(out=e_t, in_=d_t, func=AF.Exp, scale=-1.0)
                for nh in range(NH):
                    nc.tensor.matmul(
                        c2[:, nh, :],
                        tgtsT_aug[s][:, m, :],
                        e_t[:, nh * 512 : (nh + 1) * 512],
                        start=(m == 0),
                        stop=(m == MC - 1),
                    )

            # ---- per-sample tail: normalize, moment matrix, store ----
            c_sb = small.tile([4, NH, 512], fp32, tag="csb", name=f"csb{s}")
            nc.vector.tensor_copy(c_sb, c2)
            corr_all = small.tile([128, MC, 4], f32r, tag="corr", name=f"corr{s}")
            nc.vector.memset(corr_all.bitcast(fp32), 1.0)
            c_flat = c_sb.rearrange("p a b -> p (a b)")
            for q in range(MC):
                ct_ps = pss.tile([128, 4], fp32, tag="ps1", name=f"ct{s}{q}")
                nc.tensor.transpose(ct_ps, c_flat[:, q * 128 : (q + 1) * 128], ident)
                rs = small.tile([128, 1], fp32, tag="rs", name=f"rs{s}{q}")
                nc.vector.reciprocal(rs, ct_ps[:, 3:4])
                nc.vector.tensor_scalar(
                    out=corr_all[:, q, 0:3],
                    in0=ct_ps[:, 0:3],
                    scalar1=rs,
                    scalar2=None,
                    op0=mybir.AluOpType.mult,
                )
            o_ps = pss.tile([4, 4], fp32, tag="ps1", name=f"o{s}")
            for q in range(MC):
                nc.tensor.matmul(
                    o_ps,
                    srcsT_aug[s][:, q, :],
                    corr_all[:, q, :],
                    start=(q == 0),
                    stop=(q == MC - 1),
                )
            o_sb = small.tile([4, 4], fp32, tag="osb", name=f"ot{s}")
            nc.vector.tensor_copy(o_sb, o_ps)
            nc.sync.dma_start(out=out44[s], in_=o_sb)

    nc.finalize()
    _state["nc"] = nc
    return nc


def _postprocess(o44):
    """o44: [B, 4, 4] moment matrices -> [B, 6] (euler angles, translation)."""
    o = o44.astype(np.float64)
    H_raw = o[:, 0:3, 0:3]
    ssum = o[:, 0:3, 3]
    csum = o[:, 3, 0:3]
    cnt = o[:, 3, 3][:, None, None]
    H = H_raw - ssum[:, :, None] * csum[:, None, :] / cnt
    u, _, vh = np.linalg.svd(H)
    v = np.swapaxes(vh, -1, -2)
    r = v @ np.swapaxes(u, -1, -2)
    det = np.linalg.det(r)
    flip = np.where(det[:, None] < 0, np.array([1.0, 1.0, -1.0]), 1.0)
    v = v * flip[:, None, :]
    R = v @ np.swapaxes(u, -1, -2)
    sm = ssum / cnt[:, :, 0]
    cm = csum / cnt[:, :, 0]
    t = -np.einsum("bij,bj->bi", R, sm) + cm
    cy = np.sqrt(R[:, 2, 2] ** 2 + R[:, 1, 2] ** 2)
    ax = np.arctan2(-R[:, 1, 2], R[:, 2, 2])
    ay = np.arctan2(R[:, 0, 2], cy)
    az = np.arctan2(-R[:, 0, 1], R[:, 0, 0])
    return np.concatenate([np.stack([ax, ay, az], 1), t], axis=1).astype(np.float32)


def kernel(srcs, tgts, srcs_emb, tgts_emb, **run_kwargs):
    from concourse.bass_utils import run_bass_kernel_spmd

    nc = _build()
    in_maps = []
    for c in range(_NCORES):
        sl = slice(c * _SPC, (c + 1) * _SPC)
        in_maps.append(
            {
                "srcs": np.ascontiguousarray(srcs[sl], dtype=np.float32),
                "tgts": np.ascontiguousarray(tgts[sl], dtype=np.float32),
                "srcs_emb": np.ascontiguousarray(srcs_emb[sl], dtype=np.float32),
                "tgts_emb": np.ascontiguousarray(tgts_emb[sl], dtype=np.float32),
            }
        )
    res = run_bass_kernel_spmd(nc, in_maps, list(range(_NCORES)), **run_kwargs)
    o44 = np.concatenate(
        [np.asarray(res.results[c]["out44"]) for c in range(_NCORES)], axis=0
    )
    out = _postprocess(o44)
    if run_kwargs:
        _state["last_results"] = res
    return out
